# revision 22
# baseline (speedup 1.0000x reference)
"""Trainium2 Bass kernel for nn_Cat_Linear_Decoder (GNN edge-MLP decoder).

    out[r] = sigmoid( relu(cat(z[src[r]], z[dst[r]]) @ W1 + b1) @ W2
                      + b2 + sig_bias )        for r in 0..2E  (E = 500k)

Sharding: data-parallel over EDGES across 8 NeuronCores; z + MLP weights
replicated per core.  The symmetrized rows (i,j) and (j,i) of one edge are
computed on the same core from one pair of gathered node vectors, halving
gather descriptors/bytes vs row-parallel.

Device strategy per core:
  - Edges host-sorted by (src_bank, dst_bank), bank = 32768 nodes, so every
    dma_gather reads a single z bank with int16 bank-local indices.
  - z stored fp16; dma_gather(transpose=True) emits X.T tiles [128ch, edges]
    directly in SBUF -- no on-chip transposes.
  - Per 512-edge block: 4 PSUM tiles (fwd/bwd x 2 hidden chunks), 8 matmuls
    in stationary-major order (W1 quadrant loaded once per block):
        hp0f = W1a0.T@Xs + W1b0.T@Xd   hp0b = W1a0.T@Xd + W1b0.T@Xs   etc.
  - ReLU+bias PSUM->SBUF fp16 split between ACT (activation Relu, bias AP)
    and DVE (tensor_scalar add/max) to balance engines.
  - Layer 2: slice s writes PSUM partition s%128 of a [128,512] strip via a
    [128,1] W2-chunk lhsT (2 accumulating matmuls); every 128 slices one
    ACT sigmoid [128,512] (bias = b2+sig_bias) and one 256KB output DMA.
  - Outputs come back in permuted order; the host inverse-permutes.

Host-side work is restricted to sharding/permutation/packing of inputs and
the inverse permutation of the output; all FLOPs of the model run on device.
"""

import os
import sys
from contextlib import ExitStack

import numpy as np

sys.path.insert(0, "/opt/trn_rl_repo")
os.environ.setdefault("MYCRO_LOCAL_CACHE", "1")

import concourse.bacc as bacc
import concourse.mybir as mybir
import concourse.tile as tile
from concourse.bass_utils import run_bass_kernel_spmd

F16 = mybir.dt.float16
F32 = mybir.dt.float32
I16 = mybir.dt.int16

P = 128          # partitions == in_ch per side
HIDDEN = 256
W = 512          # edges per block == rows per PSUM slice
N_CORES = 8
BANK = 32768     # int16-addressable z rows per gather call
GRAN = 512       # group capacity granularity (edges; keeps all slices full)
GCHUNK = 2048    # edges per chunk (two 1024-idx gather calls per side)

# set by test.py via env to collect a perfetto trace + HW exec time
_TRACE = bool(int(os.environ.get("KERNEL_TRACE", "0")))
last_result = None  # BassKernelResults of the most recent run (for test.py)

_neff_cache = {}


def _make_plan(caps, gchunk):
    plan, off = [], 0
    for bs, bd, cap in caps:
        rem = cap
        while rem > 0:
            n = min(gchunk, rem)
            plan.append((off, n, bs, bd))
            off += n
            rem -= n
    return plan, off


def _build_kernel(v_nodes, bank, plan, e_pad):
    n_slices = 2 * e_pad // W
    nc = bacc.Bacc(num_swdge_queues=4, dynamic_dma_scratch_size=65536)
    z = nc.dram_tensor("z", [v_nodes, P], F16, kind="ExternalInput")
    si = nc.dram_tensor("si", [P, e_pad // 16], I16, kind="ExternalInput")
    di = nc.dram_tensor("di", [P, e_pad // 16], I16, kind="ExternalInput")
    w1s = nc.dram_tensor("w1s", [P, HIDDEN], F16, kind="ExternalInput")
    w1d = nc.dram_tensor("w1d", [P, HIDDEN], F16, kind="ExternalInput")
    w2 = nc.dram_tensor("w2", [P, 2], F16, kind="ExternalInput")
    b1 = nc.dram_tensor("b1", [P, 3], F32, kind="ExternalInput")
    out = nc.dram_tensor("out", [n_slices, W], F32, kind="ExternalOutput")

    with tile.TileContext(nc) as tc, ExitStack() as ctx:
        const = ctx.enter_context(tc.tile_pool(name="const", bufs=1))
        si_sb = const.tile([P, e_pad // 16], I16)
        nc.sync.dma_start(si_sb[:], si[:])
        di_sb = const.tile([P, e_pad // 16], I16)
        nc.sync.dma_start(di_sb[:], di[:])
        w1s_sb = const.tile([P, HIDDEN], F16)
        nc.sync.dma_start(w1s_sb[:], w1s[:])
        w1d_sb = const.tile([P, HIDDEN], F16)
        nc.sync.dma_start(w1d_sb[:], w1d[:])
        w2_sb = const.tile([P, 2], F16)
        nc.sync.dma_start(w2_sb[:], w2[:])
        b1_sb = const.tile([P, 3], F32)
        nc.sync.dma_start(b1_sb[:], b1[:])
        ident = const.tile([P, P], F16)
        from concourse.masks import make_identity
        make_identity(nc, ident[:])

        xpool = ctx.enter_context(tc.tile_pool(name="x", bufs=4))
        hpool = ctx.enter_context(tc.tile_pool(name="h", bufs=6))
        opool = ctx.enter_context(tc.tile_pool(name="o", bufs=2))
        pshp = ctx.enter_context(tc.tile_pool(name="pshp", bufs=4, space="PSUM"))
        pstp = ctx.enter_context(tc.tile_pool(name="pstp", bufs=2, space="PSUM"))
        psxp = ctx.enter_context(tc.tile_pool(name="psxp", bufs=2, space="PSUM"))

        state = {"sl": 0, "pend": [], "xpb": None, "grp": []}

        def close_group():
            # one sigmoid + one strided out-DMA for the <=3 strips packed at
            # PSUM partitions 0/32/64 (the legal matmul output bases)
            grp = state["grp"]
            if not grp:
                return
            nq = len(grp)
            s0 = grp[0]
            top = 32 * (nq - 1) + 1
            obt = opool.tile([P, W], F32, tag="ob", name=f"ob_{s0}")
            nc.scalar.activation(obt[:top, :], state["xpb"][:top, :],
                                 mybir.ActivationFunctionType.Sigmoid,
                                 bias=b1_sb[:top, 2:3], scale=1.0)
            obr = obt[:].rearrange("(q t) f -> q t f", t=32)
            nc.sync.dma_start(out[s0 : s0 + nq, :], obr[0:nq, 0, :])
            state["grp"] = []

        def emit_l2(h0, h1, s):
            q = s % 3
            if q == 0:
                state["xpb"] = psxp.tile([P, W], F32, tag="xp", name=f"xp_{s}")
            base = 32 * q
            xpb = state["xpb"]
            nc.tensor.matmul(xpb[base : base + 1, :], lhsT=w2_sb[:, 0:1],
                             rhs=h0[:], start=True, stop=False)
            nc.tensor.matmul(xpb[base : base + 1, :], lhsT=w2_sb[:, 1:2],
                             rhs=h1[:], start=False, stop=True)
            state["grp"].append(s)
            if q == 2 or s == n_slices - 1:
                close_group()

        def flush_pend(keep):
            while len(state["pend"]) > keep:
                emit_l2(*state["pend"].pop(0))

        qn = 0
        for off, n, bs, bd in plan:
            zs = z[bs * bank : min((bs + 1) * bank, v_nodes), :]
            zd = z[bd * bank : min((bd + 1) * bank, v_nodes), :]
            # gather row-major [row%128, row//128, ch], then one HWDGE XBAR
            # transpose per tile -> [ch, row] (transposing SWDGE gathers race)
            xsr = xpool.tile([P, n // P, P], F16, tag="xsr", name=f"xsr_{off}")
            xdr = xpool.tile([P, n // P, P], F16, tag="xdr", name=f"xdr_{off}")
            # split each gather into halves on different queues to halve its
            # latency (4 SWDGE queues, Q7 pair per queue)
            for half, (t, idx_sb) in enumerate([(xsr, si_sb), (xdr, di_sb)]):
                zb = zs if half == 0 else zd
                hn = n // 2
                for k in range(2):
                    o = off + k * hn
                    nc.gpsimd.dma_gather(
                        out_ap=t[:, k * (hn // P) : (k + 1) * (hn // P), :],
                        in_ap=zb,
                        idxs_ap=idx_sb[:, o // 16 : (o + hn) // 16],
                        num_idxs=hn, num_idxs_reg=hn, elem_size=P,
                        transpose=False, single_packet=False,
                        queue_num=(qn + 2 * half + k) % 4,
                    )
            qn += 4
            for b0 in range(0, n, W):
                # PE-transpose the 4 gathered 128-row groups of each side into
                # one fp16 PSUM bank, copy to SBUF (ACT=src half, DVE=dst half)
                xt = pstp.tile([P, 2 * W], F16, tag="xt", name=f"xt_{off}_{b0}")
                for k in range(4):
                    g = b0 // P + k
                    nc.tensor.transpose(xt[:, k * P : (k + 1) * P],
                                        xsr[:, g, :], ident[:])
                    nc.tensor.transpose(xt[:, W + k * P : W + (k + 1) * P],
                                        xdr[:, g, :], ident[:])
                xsb = xpool.tile([P, W], F16, tag="xs", name=f"xs_{off}_{b0}")
                xdb = xpool.tile([P, W], F16, tag="xd", name=f"xd_{off}_{b0}")
                nc.scalar.copy(xsb[:], xt[:, :W])
                nc.vector.tensor_scalar(out=xdb[:], in0=xt[:, W:],
                                        scalar1=0.0, scalar2=None,
                                        op0=mybir.AluOpType.add)
                # stationary-major: each W1 quadrant loaded once per block,
                # fwd/bwd accumulation groups interleaved across PSUM banks
                hp0f = pshp.tile([P, W], F32, tag="hp", name=f"hp0f_{off}_{b0}")
                hp0b = pshp.tile([P, W], F32, tag="hp", name=f"hp0b_{off}_{b0}")
                nc.tensor.matmul(hp0f[:], lhsT=w1s_sb[:, 0:128], rhs=xsb[:],
                                 start=True, stop=False)
                nc.tensor.matmul(hp0b[:], lhsT=w1s_sb[:, 0:128], rhs=xdb[:],
                                 start=True, stop=False)
                nc.tensor.matmul(hp0f[:], lhsT=w1d_sb[:, 0:128], rhs=xdb[:],
                                 start=False, stop=True)
                nc.tensor.matmul(hp0b[:], lhsT=w1d_sb[:, 0:128], rhs=xsb[:],
                                 start=False, stop=True)
                hp1f = pshp.tile([P, W], F32, tag="hp", name=f"hp1f_{off}_{b0}")
                hp1b = pshp.tile([P, W], F32, tag="hp", name=f"hp1b_{off}_{b0}")
                nc.tensor.matmul(hp1f[:], lhsT=w1s_sb[:, 128:256], rhs=xsb[:],
                                 start=True, stop=False)
                nc.tensor.matmul(hp1b[:], lhsT=w1s_sb[:, 128:256], rhs=xdb[:],
                                 start=True, stop=False)
                nc.tensor.matmul(hp1f[:], lhsT=w1d_sb[:, 128:256], rhs=xdb[:],
                                 start=False, stop=True)
                nc.tensor.matmul(hp1b[:], lhsT=w1d_sb[:, 128:256], rhs=xsb[:],
                                 start=False, stop=True)
                for tg, hp0, hp1 in (("f", hp0f, hp1f), ("b", hp0b, hp1b)):
                    h0 = hpool.tile([P, W], F16, tag="h0", name=f"h0{tg}_{off}_{b0}")
                    h1 = hpool.tile([P, W], F16, tag="h1", name=f"h1{tg}_{off}_{b0}")
                    nc.scalar.activation(h0[:], hp0[:],
                                         mybir.ActivationFunctionType.Relu,
                                         bias=b1_sb[:, 0:1], scale=1.0)
                    nc.vector.tensor_scalar(out=h1[:], in0=hp1[:],
                                            scalar1=b1_sb[:, 1:2], scalar2=0.0,
                                            op0=mybir.AluOpType.add,
                                            op1=mybir.AluOpType.max)
                    state["pend"].append((h0, h1, state["sl"]))
                    state["sl"] += 1
                    # defer L2 ~2 slices so relu has drained before PE needs it
                    flush_pend(2)
        flush_pend(0)
    nc.compile()
    return nc


def _pack_idx16(idx, e_pad):
    """int32 [e_pad] -> int16 [128, e_pad//16] wrapped+replicated layout."""
    t = idx.astype(np.int16).reshape(e_pad // 16, 16).T
    return np.ascontiguousarray(np.tile(t, (8, 1)))


def kernel(z, edge_index, W1, b1, W2, b2, sig_bias):
    global last_result
    z = np.asarray(z)
    edge_index = np.asarray(edge_index)
    W1 = np.asarray(W1, dtype=np.float32)
    b1 = np.asarray(b1, dtype=np.float32)
    W2 = np.asarray(W2, dtype=np.float32)
    b2 = np.asarray(b2, dtype=np.float32)
    sig_bias = np.asarray(sig_bias, dtype=np.float32)

    v = z.shape[0]
    e = edge_index.shape[1]
    r = 2 * e
    per = e // N_CORES
    nb = (v + BANK - 1) // BANK

    ei0 = edge_index[0].astype(np.int32)
    ei1 = edge_index[1].astype(np.int32)

    # per-core grouping of EDGES by (src_bank, dst_bank)
    per_core = []
    counts_all = np.zeros((N_CORES, nb * nb), dtype=np.int64)
    for c in range(N_CORES):
        s = ei0[c * per : (c + 1) * per]
        d = ei1[c * per : (c + 1) * per]
        gid = (s // BANK) * nb + (d // BANK)
        order = np.argsort(gid, kind="stable")
        counts = np.bincount(gid, minlength=nb * nb)
        counts_all[c] = counts
        per_core.append((s, d, order, counts))

    maxc = counts_all.max(axis=0)
    caps = []
    for g in range(nb * nb):
        if maxc[g] == 0:
            continue
        caps.append((g // nb, g % nb, int(-(-maxc[g] // GRAN) * GRAN)))
    plan, e_pad = _make_plan(caps, GCHUNK)
    m_pad = 2 * e_pad

    zf = np.ascontiguousarray(z.astype(np.float16))
    w1s = np.ascontiguousarray(W1[:P, :].astype(np.float16))
    w1d = np.ascontiguousarray(W1[P:, :].astype(np.float16))
    w2p = np.ascontiguousarray(
        np.stack([W2[:P, 0], W2[P:, 0]], axis=1).astype(np.float16))
    bias2 = float(np.float32(b2[0]) + np.float32(sig_bias[0]))
    b1p = np.ascontiguousarray(
        np.stack([b1[:P], b1[P:], np.full(P, bias2)], axis=1).astype(np.float32))

    # padded-edge-position -> output row: block k = p//512 of 512 edges emits
    # fwd rows [1024k, 1024k+512) then bwd rows [1024k+512, 1024k+1024)
    in_maps = []
    orig_rows = []
    for c in range(N_CORES):
        s, d, order, counts = per_core[c]
        sp = np.zeros(e_pad, dtype=np.int32)
        dp = np.zeros(e_pad, dtype=np.int32)
        oge = np.full(e_pad, -1, dtype=np.int64)  # global edge id per padded pos
        cum = np.concatenate([[0], np.cumsum(counts)])
        off = 0
        for bs, bd, cap in caps:
            g = bs * nb + bd
            cnt = int(counts[g])
            rows = order[cum[g] : cum[g] + cnt]
            sp[off : off + cnt] = s[rows] - bs * BANK
            dp[off : off + cnt] = d[rows] - bd * BANK
            oge[off : off + cnt] = c * per + rows
            off += cap
        in_maps.append({
            "z": zf,
            "si": _pack_idx16(sp, e_pad),
            "di": _pack_idx16(dp, e_pad),
            "w1s": w1s, "w1d": w1d, "w2": w2p, "b1": b1p,
        })
        # device row for padded edge pos p: fwd = 1024*(p//512) + p%512
        pidx = np.arange(e_pad, dtype=np.int64)
        fwd_rows = 1024 * (pidx // 512) + pidx % 512
        og = np.full(m_pad, -1, dtype=np.int64)
        m = oge >= 0
        og[fwd_rows[m]] = oge[m]            # row ids 0..E-1
        og[fwd_rows[m] + 512] = e + oge[m]  # row ids E..2E-1
        orig_rows.append(og)

    key = (v, e_pad, tuple(plan))
    if key not in _neff_cache:
        _neff_cache[key] = _build_kernel(v, BANK, plan, e_pad)
    nc = _neff_cache[key]

    res = run_bass_kernel_spmd(nc, in_maps, list(range(N_CORES)), trace=_TRACE)
    last_result = res

    result = np.zeros(r, dtype=np.float32)
    for o, og in zip(res.results, orig_rows):
        m = og >= 0
        result[og[m]] = np.asarray(o["out"], dtype=np.float32).ravel()[m]
    return result


# revision 23
# speedup vs baseline: 1.0120x; 1.0120x over previous
"""Trainium2 Bass kernel for nn_Cat_Linear_Decoder (GNN edge-MLP decoder).

    out[r] = sigmoid( relu(cat(z[src[r]], z[dst[r]]) @ W1 + b1) @ W2
                      + b2 + sig_bias )        for r in 0..2E  (E = 500k)

Sharding: data-parallel over EDGES across 8 NeuronCores; z + MLP weights
replicated per core.  The symmetrized rows (i,j) and (j,i) of one edge are
computed on the same core from one pair of gathered node vectors, halving
gather descriptors/bytes vs row-parallel.

Device strategy per core:
  - Edges host-sorted by (src_bank, dst_bank), bank = 32768 nodes, so every
    dma_gather reads a single z bank with int16 bank-local indices.
  - z stored fp16; dma_gather(transpose=True) emits X.T tiles [128ch, edges]
    directly in SBUF -- no on-chip transposes.
  - Per 512-edge block: 4 PSUM tiles (fwd/bwd x 2 hidden chunks), 8 matmuls
    in stationary-major order (W1 quadrant loaded once per block):
        hp0f = W1a0.T@Xs + W1b0.T@Xd   hp0b = W1a0.T@Xd + W1b0.T@Xs   etc.
  - ReLU+bias PSUM->SBUF fp16 split between ACT (activation Relu, bias AP)
    and DVE (tensor_scalar add/max) to balance engines.
  - Layer 2: slice s writes PSUM partition s%128 of a [128,512] strip via a
    [128,1] W2-chunk lhsT (2 accumulating matmuls); every 128 slices one
    ACT sigmoid [128,512] (bias = b2+sig_bias) and one 256KB output DMA.
  - Outputs come back in permuted order; the host inverse-permutes.

Host-side work is restricted to sharding/permutation/packing of inputs and
the inverse permutation of the output; all FLOPs of the model run on device.
"""

import os
import sys
from contextlib import ExitStack

import numpy as np

sys.path.insert(0, "/opt/trn_rl_repo")
os.environ.setdefault("MYCRO_LOCAL_CACHE", "1")

import concourse.bacc as bacc
import concourse.mybir as mybir
import concourse.tile as tile
from concourse.bass_utils import run_bass_kernel_spmd

F16 = mybir.dt.float16
F32 = mybir.dt.float32
I16 = mybir.dt.int16

P = 128          # partitions == in_ch per side
HIDDEN = 256
W = 512          # edges per block == rows per PSUM slice
N_CORES = 8
BANK = 32768     # int16-addressable z rows per gather call
GRAN = 512       # group capacity granularity (edges; keeps all slices full)
GCHUNK = 2048    # edges per chunk (two 1024-idx gather calls per side)

# set by test.py via env to collect a perfetto trace + HW exec time
_TRACE = bool(int(os.environ.get("KERNEL_TRACE", "0")))
last_result = None  # BassKernelResults of the most recent run (for test.py)

_neff_cache = {}


def _make_plan(caps, gchunk):
    plan, off = [], 0
    for bs, bd, cap in caps:
        rem = cap
        while rem > 0:
            n = min(gchunk, rem)
            plan.append((off, n, bs, bd))
            off += n
            rem -= n
    return plan, off


def _build_kernel(v_nodes, bank, plan, e_pad):
    n_slices = 2 * e_pad // W
    nc = bacc.Bacc(num_swdge_queues=4, dynamic_dma_scratch_size=65536)
    z = nc.dram_tensor("z", [v_nodes, P], F16, kind="ExternalInput")
    si = nc.dram_tensor("si", [P, e_pad // 16], I16, kind="ExternalInput")
    di = nc.dram_tensor("di", [P, e_pad // 16], I16, kind="ExternalInput")
    w1s = nc.dram_tensor("w1s", [P, HIDDEN], F16, kind="ExternalInput")
    w1d = nc.dram_tensor("w1d", [P, HIDDEN], F16, kind="ExternalInput")
    w2 = nc.dram_tensor("w2", [P, 2], F16, kind="ExternalInput")
    b1 = nc.dram_tensor("b1", [P, 3], F32, kind="ExternalInput")
    out = nc.dram_tensor("out", [n_slices, W], F32, kind="ExternalOutput")

    with tile.TileContext(nc) as tc, ExitStack() as ctx:
        const = ctx.enter_context(tc.tile_pool(name="const", bufs=1))
        si_sb = const.tile([P, e_pad // 16], I16)
        nc.sync.dma_start(si_sb[:], si[:])
        di_sb = const.tile([P, e_pad // 16], I16)
        nc.sync.dma_start(di_sb[:], di[:])
        w1s_sb = const.tile([P, HIDDEN], F16)
        nc.sync.dma_start(w1s_sb[:], w1s[:])
        w1d_sb = const.tile([P, HIDDEN], F16)
        nc.sync.dma_start(w1d_sb[:], w1d[:])
        w2_sb = const.tile([P, 2], F16)
        nc.sync.dma_start(w2_sb[:], w2[:])
        b1_sb = const.tile([P, 3], F32)
        nc.sync.dma_start(b1_sb[:], b1[:])
        ident = const.tile([P, P], F16)
        from concourse.masks import make_identity
        make_identity(nc, ident[:])

        xpool = ctx.enter_context(tc.tile_pool(name="x", bufs=4))
        hpool = ctx.enter_context(tc.tile_pool(name="h", bufs=6))
        opool = ctx.enter_context(tc.tile_pool(name="o", bufs=2))
        pshp = ctx.enter_context(tc.tile_pool(name="pshp", bufs=4, space="PSUM"))
        pstp = ctx.enter_context(tc.tile_pool(name="pstp", bufs=2, space="PSUM"))
        psxp = ctx.enter_context(tc.tile_pool(name="psxp", bufs=2, space="PSUM"))

        state = {"sl": 0, "pend": [], "xpb": None, "grp": []}

        def close_group():
            # one sigmoid + one strided out-DMA for the <=3 strips packed at
            # PSUM partitions 0/32/64 (the legal matmul output bases)
            grp = state["grp"]
            if not grp:
                return
            nq = len(grp)
            s0 = grp[0]
            top = 32 * (nq - 1) + 1
            obt = opool.tile([P, W], F32, tag="ob", name=f"ob_{s0}")
            nc.scalar.activation(obt[:top, :], state["xpb"][:top, :],
                                 mybir.ActivationFunctionType.Sigmoid,
                                 bias=b1_sb[:top, 2:3], scale=1.0)
            obr = obt[:].rearrange("(q t) f -> q t f", t=32)
            nc.sync.dma_start(out[s0 : s0 + nq, :], obr[0:nq, 0, :])
            state["grp"] = []

        def emit_l2(h0, h1, s):
            q = s % 3
            if q == 0:
                state["xpb"] = psxp.tile([P, W], F32, tag="xp", name=f"xp_{s}")
            base = 32 * q
            xpb = state["xpb"]
            nc.tensor.matmul(xpb[base : base + 1, :], lhsT=w2_sb[:, 0:1],
                             rhs=h0[:], start=True, stop=False)
            nc.tensor.matmul(xpb[base : base + 1, :], lhsT=w2_sb[:, 1:2],
                             rhs=h1[:], start=False, stop=True)
            state["grp"].append(s)
            if q == 2 or s == n_slices - 1:
                close_group()

        def flush_pend(keep):
            while len(state["pend"]) > keep:
                emit_l2(*state["pend"].pop(0))

        qn = 0
        for off, n, bs, bd in plan:
            zs = z[bs * bank : min((bs + 1) * bank, v_nodes), :]
            zd = z[bd * bank : min((bd + 1) * bank, v_nodes), :]
            # gather row-major [row%128, row//128, ch], then one HWDGE XBAR
            # transpose per tile -> [ch, row] (transposing SWDGE gathers race)
            xsr = xpool.tile([P, n // P, P], F16, tag="xsr", name=f"xsr_{off}")
            xdr = xpool.tile([P, n // P, P], F16, tag="xdr", name=f"xdr_{off}")
            # split each gather into halves on different queues to halve its
            # latency (4 SWDGE queues, Q7 pair per queue)
            for half, (t, idx_sb) in enumerate([(xsr, si_sb), (xdr, di_sb)]):
                zb = zs if half == 0 else zd
                hn = n // 2
                for k in range(2):
                    o = off + k * hn
                    nc.gpsimd.dma_gather(
                        out_ap=t[:, k * (hn // P) : (k + 1) * (hn // P), :],
                        in_ap=zb,
                        idxs_ap=idx_sb[:, o // 16 : (o + hn) // 16],
                        num_idxs=hn, num_idxs_reg=hn, elem_size=P,
                        transpose=False, single_packet=False,
                        queue_num=(qn + 2 * half + k) % 4,
                    )
            qn += 4
            for b0 in range(0, n, W):
                # PE-transpose the 4 gathered 128-row groups of each side into
                # one fp16 PSUM bank, copy to SBUF (ACT=src half, DVE=dst half)
                xt = pstp.tile([P, 2 * W], F16, tag="xt", name=f"xt_{off}_{b0}")
                for k in range(4):
                    g = b0 // P + k
                    nc.tensor.transpose(xt[:, k * P : (k + 1) * P],
                                        xsr[:, g, :], ident[:])
                    nc.tensor.transpose(xt[:, W + k * P : W + (k + 1) * P],
                                        xdr[:, g, :], ident[:])
                xsb = xpool.tile([P, W], F16, tag="xs", name=f"xs_{off}_{b0}")
                xdb = xpool.tile([P, W], F16, tag="xd", name=f"xd_{off}_{b0}")
                nc.scalar.copy(xsb[:], xt[:, :W])
                nc.vector.tensor_scalar(out=xdb[:], in0=xt[:, W:],
                                        scalar1=0.0, scalar2=None,
                                        op0=mybir.AluOpType.add)
                for fwd in (True, False):
                    ra, rb = (xsb, xdb) if fwd else (xdb, xsb)
                    tg = "f" if fwd else "b"
                    hp0 = pshp.tile([P, W], F32, tag="hp", name=f"hp0{tg}_{off}_{b0}")
                    hp1 = pshp.tile([P, W], F32, tag="hp", name=f"hp1{tg}_{off}_{b0}")
                    nc.tensor.matmul(hp0[:], lhsT=w1s_sb[:, 0:128], rhs=ra[:],
                                     start=True, stop=False)
                    nc.tensor.matmul(hp0[:], lhsT=w1d_sb[:, 0:128], rhs=rb[:],
                                     start=False, stop=True)
                    nc.tensor.matmul(hp1[:], lhsT=w1s_sb[:, 128:256], rhs=ra[:],
                                     start=True, stop=False)
                    nc.tensor.matmul(hp1[:], lhsT=w1d_sb[:, 128:256], rhs=rb[:],
                                     start=False, stop=True)
                    h0 = hpool.tile([P, W], F16, tag="h0", name=f"h0{tg}_{off}_{b0}")
                    h1 = hpool.tile([P, W], F16, tag="h1", name=f"h1{tg}_{off}_{b0}")
                    nc.scalar.activation(h0[:], hp0[:],
                                         mybir.ActivationFunctionType.Relu,
                                         bias=b1_sb[:, 0:1], scale=1.0)
                    nc.vector.tensor_scalar(out=h1[:], in0=hp1[:],
                                            scalar1=b1_sb[:, 1:2], scalar2=0.0,
                                            op0=mybir.AluOpType.add,
                                            op1=mybir.AluOpType.max)
                    state["pend"].append((h0, h1, state["sl"]))
                    state["sl"] += 1
                    # defer L2 ~2 slices so relu has drained before PE needs it
                    flush_pend(2)
        flush_pend(0)
    nc.compile()
    return nc


def _pack_idx16(idx, e_pad):
    """int32 [e_pad] -> int16 [128, e_pad//16] wrapped+replicated layout."""
    t = idx.astype(np.int16).reshape(e_pad // 16, 16).T
    return np.ascontiguousarray(np.tile(t, (8, 1)))


def kernel(z, edge_index, W1, b1, W2, b2, sig_bias):
    global last_result
    z = np.asarray(z)
    edge_index = np.asarray(edge_index)
    W1 = np.asarray(W1, dtype=np.float32)
    b1 = np.asarray(b1, dtype=np.float32)
    W2 = np.asarray(W2, dtype=np.float32)
    b2 = np.asarray(b2, dtype=np.float32)
    sig_bias = np.asarray(sig_bias, dtype=np.float32)

    v = z.shape[0]
    e = edge_index.shape[1]
    r = 2 * e
    per = e // N_CORES
    nb = (v + BANK - 1) // BANK

    ei0 = edge_index[0].astype(np.int32)
    ei1 = edge_index[1].astype(np.int32)

    # per-core grouping of EDGES by (src_bank, dst_bank)
    per_core = []
    counts_all = np.zeros((N_CORES, nb * nb), dtype=np.int64)
    for c in range(N_CORES):
        s = ei0[c * per : (c + 1) * per]
        d = ei1[c * per : (c + 1) * per]
        gid = (s // BANK) * nb + (d // BANK)
        order = np.argsort(gid, kind="stable")
        counts = np.bincount(gid, minlength=nb * nb)
        counts_all[c] = counts
        per_core.append((s, d, order, counts))

    maxc = counts_all.max(axis=0)
    caps = []
    for g in range(nb * nb):
        if maxc[g] == 0:
            continue
        caps.append((g // nb, g % nb, int(-(-maxc[g] // GRAN) * GRAN)))
    plan, e_pad = _make_plan(caps, GCHUNK)
    m_pad = 2 * e_pad

    zf = np.ascontiguousarray(z.astype(np.float16))
    w1s = np.ascontiguousarray(W1[:P, :].astype(np.float16))
    w1d = np.ascontiguousarray(W1[P:, :].astype(np.float16))
    w2p = np.ascontiguousarray(
        np.stack([W2[:P, 0], W2[P:, 0]], axis=1).astype(np.float16))
    bias2 = float(np.float32(b2[0]) + np.float32(sig_bias[0]))
    b1p = np.ascontiguousarray(
        np.stack([b1[:P], b1[P:], np.full(P, bias2)], axis=1).astype(np.float32))

    # padded-edge-position -> output row: block k = p//512 of 512 edges emits
    # fwd rows [1024k, 1024k+512) then bwd rows [1024k+512, 1024k+1024)
    in_maps = []
    orig_rows = []
    for c in range(N_CORES):
        s, d, order, counts = per_core[c]
        sp = np.zeros(e_pad, dtype=np.int32)
        dp = np.zeros(e_pad, dtype=np.int32)
        oge = np.full(e_pad, -1, dtype=np.int64)  # global edge id per padded pos
        cum = np.concatenate([[0], np.cumsum(counts)])
        off = 0
        for bs, bd, cap in caps:
            g = bs * nb + bd
            cnt = int(counts[g])
            rows = order[cum[g] : cum[g] + cnt]
            sp[off : off + cnt] = s[rows] - bs * BANK
            dp[off : off + cnt] = d[rows] - bd * BANK
            oge[off : off + cnt] = c * per + rows
            off += cap
        in_maps.append({
            "z": zf,
            "si": _pack_idx16(sp, e_pad),
            "di": _pack_idx16(dp, e_pad),
            "w1s": w1s, "w1d": w1d, "w2": w2p, "b1": b1p,
        })
        # device row for padded edge pos p: fwd = 1024*(p//512) + p%512
        pidx = np.arange(e_pad, dtype=np.int64)
        fwd_rows = 1024 * (pidx // 512) + pidx % 512
        og = np.full(m_pad, -1, dtype=np.int64)
        m = oge >= 0
        og[fwd_rows[m]] = oge[m]            # row ids 0..E-1
        og[fwd_rows[m] + 512] = e + oge[m]  # row ids E..2E-1
        orig_rows.append(og)

    key = (v, e_pad, tuple(plan))
    if key not in _neff_cache:
        _neff_cache[key] = _build_kernel(v, BANK, plan, e_pad)
    nc = _neff_cache[key]

    res = run_bass_kernel_spmd(nc, in_maps, list(range(N_CORES)), trace=_TRACE)
    last_result = res

    result = np.zeros(r, dtype=np.float32)
    for o, og in zip(res.results, orig_rows):
        m = og >= 0
        result[og[m]] = np.asarray(o["out"], dtype=np.float32).ravel()[m]
    return result


# revision 24
# speedup vs baseline: 1.1067x; 1.0936x over previous
"""Trainium2 Bass kernel for nn_Cat_Linear_Decoder (GNN edge-MLP decoder).

    out[r] = sigmoid( relu(cat(z[src[r]], z[dst[r]]) @ W1 + b1) @ W2
                      + b2 + sig_bias )        for r in 0..2E  (E = 500k)

Sharding: data-parallel over EDGES across 8 NeuronCores; z + MLP weights
replicated per core.  The symmetrized rows (i,j) and (j,i) of one edge are
computed on the same core from one pair of gathered node vectors, halving
gather descriptors/bytes vs row-parallel.

Device strategy per core:
  - Edges host-sorted by (src_bank, dst_bank), bank = 32768 nodes, so every
    dma_gather reads a single z bank with int16 bank-local indices.
  - z stored fp16; dma_gather(transpose=True) emits X.T tiles [128ch, edges]
    directly in SBUF -- no on-chip transposes.
  - Per 512-edge block: 4 PSUM tiles (fwd/bwd x 2 hidden chunks), 8 matmuls
    in stationary-major order (W1 quadrant loaded once per block):
        hp0f = W1a0.T@Xs + W1b0.T@Xd   hp0b = W1a0.T@Xd + W1b0.T@Xs   etc.
  - ReLU+bias PSUM->SBUF fp16 split between ACT (activation Relu, bias AP)
    and DVE (tensor_scalar add/max) to balance engines.
  - Layer 2: slice s writes PSUM partition s%128 of a [128,512] strip via a
    [128,1] W2-chunk lhsT (2 accumulating matmuls); every 128 slices one
    ACT sigmoid [128,512] (bias = b2+sig_bias) and one 256KB output DMA.
  - Outputs come back in permuted order; the host inverse-permutes.

Host-side work is restricted to sharding/permutation/packing of inputs and
the inverse permutation of the output; all FLOPs of the model run on device.
"""

import os
import sys
from contextlib import ExitStack

import numpy as np

sys.path.insert(0, "/opt/trn_rl_repo")
os.environ.setdefault("MYCRO_LOCAL_CACHE", "1")

import concourse.bacc as bacc
import concourse.mybir as mybir
import concourse.tile as tile
from concourse.bass_utils import run_bass_kernel_spmd

F16 = mybir.dt.float16
F32 = mybir.dt.float32
I16 = mybir.dt.int16

P = 128          # partitions == in_ch per side
HIDDEN = 256
W = 512          # edges per block == rows per PSUM slice
N_CORES = 8
BANK = 32768     # int16-addressable z rows per gather call
GRAN = 128       # group capacity granularity (edges)
GCHUNK = 2048    # edges per chunk (gathers split per bank-group segment)

# set by test.py via env to collect a perfetto trace + HW exec time
_TRACE = bool(int(os.environ.get("KERNEL_TRACE", "0")))
last_result = None  # BassKernelResults of the most recent run (for test.py)

_neff_cache = {}


def _make_plan(caps, gchunk):
    """Chunks of <=gchunk edges (multiples of W); each chunk lists its
    per-bank-group gather segments (seg_off_in_chunk, seg_len, bs, bd)."""
    e_pad = sum(c for _, _, c in caps)
    e_pad = -(-e_pad // W) * W
    # group spans in the padded edge stream
    spans, off = [], 0
    for bs, bd, cap in caps:
        spans.append((off, off + cap, bs, bd))
        off += cap
    if off < e_pad:  # filler rides the last group's banks
        s0, _, bs, bd = spans[-1]
        spans[-1] = (s0, e_pad, bs, bd)
    plan = []
    for c0 in range(0, e_pad, gchunk):
        c1 = min(c0 + gchunk, e_pad)
        segs = []
        for g0, g1, bs, bd in spans:
            a, b = max(g0, c0), min(g1, c1)
            if a < b:
                segs.append((a - c0, b - a, bs, bd))
        plan.append((c0, c1 - c0, tuple(segs)))
    return plan, e_pad


def _build_kernel(v_nodes, bank, plan, e_pad):
    n_slices = 2 * e_pad // W
    nc = bacc.Bacc(num_swdge_queues=4, dynamic_dma_scratch_size=65536)
    z = nc.dram_tensor("z", [v_nodes, P], F16, kind="ExternalInput")
    si = nc.dram_tensor("si", [P, e_pad // 16], I16, kind="ExternalInput")
    di = nc.dram_tensor("di", [P, e_pad // 16], I16, kind="ExternalInput")
    w1s = nc.dram_tensor("w1s", [P, HIDDEN], F16, kind="ExternalInput")
    w1d = nc.dram_tensor("w1d", [P, HIDDEN], F16, kind="ExternalInput")
    w2 = nc.dram_tensor("w2", [P, 2], F16, kind="ExternalInput")
    b1 = nc.dram_tensor("b1", [P, 3], F32, kind="ExternalInput")
    out = nc.dram_tensor("out", [n_slices, W], F32, kind="ExternalOutput")

    with tile.TileContext(nc) as tc, ExitStack() as ctx:
        const = ctx.enter_context(tc.tile_pool(name="const", bufs=1))
        si_sb = const.tile([P, e_pad // 16], I16)
        nc.sync.dma_start(si_sb[:], si[:])
        di_sb = const.tile([P, e_pad // 16], I16)
        nc.sync.dma_start(di_sb[:], di[:])
        w1s_sb = const.tile([P, HIDDEN], F16)
        nc.sync.dma_start(w1s_sb[:], w1s[:])
        w1d_sb = const.tile([P, HIDDEN], F16)
        nc.sync.dma_start(w1d_sb[:], w1d[:])
        w2_sb = const.tile([P, 2], F16)
        nc.sync.dma_start(w2_sb[:], w2[:])
        b1_sb = const.tile([P, 3], F32)
        nc.sync.dma_start(b1_sb[:], b1[:])
        ident = const.tile([P, P], F16)
        from concourse.masks import make_identity
        make_identity(nc, ident[:])

        xpool = ctx.enter_context(tc.tile_pool(name="x", bufs=4))
        hpool = ctx.enter_context(tc.tile_pool(name="h", bufs=6))
        opool = ctx.enter_context(tc.tile_pool(name="o", bufs=2))
        pshp = ctx.enter_context(tc.tile_pool(name="pshp", bufs=4, space="PSUM"))
        pstp = ctx.enter_context(tc.tile_pool(name="pstp", bufs=2, space="PSUM"))
        psxp = ctx.enter_context(tc.tile_pool(name="psxp", bufs=2, space="PSUM"))

        state = {"sl": 0, "pend": [], "xpb": None, "grp": []}

        def close_group():
            # one sigmoid + one strided out-DMA for the <=3 strips packed at
            # PSUM partitions 0/32/64 (the legal matmul output bases)
            grp = state["grp"]
            if not grp:
                return
            nq = len(grp)
            s0 = grp[0]
            top = 32 * (nq - 1) + 1
            obt = opool.tile([P, W], F32, tag="ob", name=f"ob_{s0}")
            nc.scalar.activation(obt[:top, :], state["xpb"][:top, :],
                                 mybir.ActivationFunctionType.Sigmoid,
                                 bias=b1_sb[:top, 2:3], scale=1.0)
            obr = obt[:].rearrange("(q t) f -> q t f", t=32)
            nc.sync.dma_start(out[s0 : s0 + nq, :], obr[0:nq, 0, :])
            state["grp"] = []

        def emit_l2(h0, h1, s):
            q = s % 3
            if q == 0:
                state["xpb"] = psxp.tile([P, W], F32, tag="xp", name=f"xp_{s}")
            base = 32 * q
            xpb = state["xpb"]
            nc.tensor.matmul(xpb[base : base + 1, :], lhsT=w2_sb[:, 0:1],
                             rhs=h0[:], start=True, stop=False)
            nc.tensor.matmul(xpb[base : base + 1, :], lhsT=w2_sb[:, 1:2],
                             rhs=h1[:], start=False, stop=True)
            state["grp"].append(s)
            if q == 2 or s == n_slices - 1:
                close_group()

        def flush_pend(keep):
            while len(state["pend"]) > keep:
                emit_l2(*state["pend"].pop(0))

        qn = 0
        for off, n, segs in plan:
            # gather row-major [row%128, row//128, ch]; one call per
            # (bank-group segment, side), round-robin over the 4 SWDGE queues
            xsr = xpool.tile([P, n // P, P], F16, tag="xsr", name=f"xsr_{off}")
            xdr = xpool.tile([P, n // P, P], F16, tag="xdr", name=f"xdr_{off}")
            for so, sl, bs, bd in segs:
                for t, idx_sb, bk in ((xsr, si_sb, bs), (xdr, di_sb, bd)):
                    # halve long segments across two queues for latency
                    parts = 2 if sl >= 1024 else 1
                    pl = sl // parts
                    pl -= pl % P
                    for k in range(parts):
                        a = so + k * pl
                        ln = pl if k < parts - 1 else sl - (parts - 1) * pl
                        nc.gpsimd.dma_gather(
                            out_ap=t[:, a // P : (a + ln) // P, :],
                            in_ap=z[bk * bank : min((bk + 1) * bank, v_nodes), :],
                            idxs_ap=idx_sb[:, (off + a) // 16 : (off + a + ln) // 16],
                            num_idxs=ln, num_idxs_reg=ln, elem_size=P,
                            transpose=False, single_packet=False,
                            queue_num=qn % 4,
                        )
                        qn += 1
            for b0 in range(0, n, W):
                # PE-transpose the 4 gathered 128-row groups of each side into
                # one fp16 PSUM bank, copy to SBUF (ACT=src half, DVE=dst half)
                xt = pstp.tile([P, 2 * W], F16, tag="xt", name=f"xt_{off}_{b0}")
                for k in range(4):
                    g = b0 // P + k
                    nc.tensor.transpose(xt[:, k * P : (k + 1) * P],
                                        xsr[:, g, :], ident[:])
                    nc.tensor.transpose(xt[:, W + k * P : W + (k + 1) * P],
                                        xdr[:, g, :], ident[:])
                xsb = xpool.tile([P, W], F16, tag="xs", name=f"xs_{off}_{b0}")
                xdb = xpool.tile([P, W], F16, tag="xd", name=f"xd_{off}_{b0}")
                nc.scalar.copy(xsb[:], xt[:, :W])
                nc.vector.tensor_scalar(out=xdb[:], in0=xt[:, W:],
                                        scalar1=0.0, scalar2=None,
                                        op0=mybir.AluOpType.add)
                for fwd in (True, False):
                    ra, rb = (xsb, xdb) if fwd else (xdb, xsb)
                    tg = "f" if fwd else "b"
                    hp0 = pshp.tile([P, W], F32, tag="hp", name=f"hp0{tg}_{off}_{b0}")
                    hp1 = pshp.tile([P, W], F32, tag="hp", name=f"hp1{tg}_{off}_{b0}")
                    nc.tensor.matmul(hp0[:], lhsT=w1s_sb[:, 0:128], rhs=ra[:],
                                     start=True, stop=False)
                    nc.tensor.matmul(hp0[:], lhsT=w1d_sb[:, 0:128], rhs=rb[:],
                                     start=False, stop=True)
                    nc.tensor.matmul(hp1[:], lhsT=w1s_sb[:, 128:256], rhs=ra[:],
                                     start=True, stop=False)
                    nc.tensor.matmul(hp1[:], lhsT=w1d_sb[:, 128:256], rhs=rb[:],
                                     start=False, stop=True)
                    h0 = hpool.tile([P, W], F16, tag="h0", name=f"h0{tg}_{off}_{b0}")
                    h1 = hpool.tile([P, W], F16, tag="h1", name=f"h1{tg}_{off}_{b0}")
                    nc.scalar.activation(h0[:], hp0[:],
                                         mybir.ActivationFunctionType.Relu,
                                         bias=b1_sb[:, 0:1], scale=1.0)
                    nc.vector.tensor_scalar(out=h1[:], in0=hp1[:],
                                            scalar1=b1_sb[:, 1:2], scalar2=0.0,
                                            op0=mybir.AluOpType.add,
                                            op1=mybir.AluOpType.max)
                    state["pend"].append((h0, h1, state["sl"]))
                    state["sl"] += 1
                    # defer L2 ~2 slices so relu has drained before PE needs it
                    flush_pend(2)
        flush_pend(0)
    nc.compile()
    return nc


def _pack_idx16(idx, e_pad):
    """int32 [e_pad] -> int16 [128, e_pad//16] wrapped+replicated layout."""
    t = idx.astype(np.int16).reshape(e_pad // 16, 16).T
    return np.ascontiguousarray(np.tile(t, (8, 1)))


def kernel(z, edge_index, W1, b1, W2, b2, sig_bias):
    global last_result
    z = np.asarray(z)
    edge_index = np.asarray(edge_index)
    W1 = np.asarray(W1, dtype=np.float32)
    b1 = np.asarray(b1, dtype=np.float32)
    W2 = np.asarray(W2, dtype=np.float32)
    b2 = np.asarray(b2, dtype=np.float32)
    sig_bias = np.asarray(sig_bias, dtype=np.float32)

    v = z.shape[0]
    e = edge_index.shape[1]
    r = 2 * e
    per = e // N_CORES
    nb = (v + BANK - 1) // BANK

    ei0 = edge_index[0].astype(np.int32)
    ei1 = edge_index[1].astype(np.int32)

    # per-core grouping of EDGES by (src_bank, dst_bank)
    per_core = []
    counts_all = np.zeros((N_CORES, nb * nb), dtype=np.int64)
    for c in range(N_CORES):
        s = ei0[c * per : (c + 1) * per]
        d = ei1[c * per : (c + 1) * per]
        gid = (s // BANK) * nb + (d // BANK)
        order = np.argsort(gid, kind="stable")
        counts = np.bincount(gid, minlength=nb * nb)
        counts_all[c] = counts
        per_core.append((s, d, order, counts))

    maxc = counts_all.max(axis=0)
    caps = []
    for g in range(nb * nb):
        if maxc[g] == 0:
            continue
        caps.append((g // nb, g % nb, int(-(-maxc[g] // GRAN) * GRAN)))
    plan, e_pad = _make_plan(caps, GCHUNK)
    m_pad = 2 * e_pad

    zf = np.ascontiguousarray(z.astype(np.float16))
    w1s = np.ascontiguousarray(W1[:P, :].astype(np.float16))
    w1d = np.ascontiguousarray(W1[P:, :].astype(np.float16))
    w2p = np.ascontiguousarray(
        np.stack([W2[:P, 0], W2[P:, 0]], axis=1).astype(np.float16))
    bias2 = float(np.float32(b2[0]) + np.float32(sig_bias[0]))
    b1p = np.ascontiguousarray(
        np.stack([b1[:P], b1[P:], np.full(P, bias2)], axis=1).astype(np.float32))

    # padded-edge-position -> output row: block k = p//512 of 512 edges emits
    # fwd rows [1024k, 1024k+512) then bwd rows [1024k+512, 1024k+1024)
    in_maps = []
    orig_rows = []
    for c in range(N_CORES):
        s, d, order, counts = per_core[c]
        sp = np.zeros(e_pad, dtype=np.int32)
        dp = np.zeros(e_pad, dtype=np.int32)
        oge = np.full(e_pad, -1, dtype=np.int64)  # global edge id per padded pos
        cum = np.concatenate([[0], np.cumsum(counts)])
        off = 0
        for bs, bd, cap in caps:
            g = bs * nb + bd
            cnt = int(counts[g])
            rows = order[cum[g] : cum[g] + cnt]
            sp[off : off + cnt] = s[rows] - bs * BANK
            dp[off : off + cnt] = d[rows] - bd * BANK
            oge[off : off + cnt] = c * per + rows
            off += cap
        in_maps.append({
            "z": zf,
            "si": _pack_idx16(sp, e_pad),
            "di": _pack_idx16(dp, e_pad),
            "w1s": w1s, "w1d": w1d, "w2": w2p, "b1": b1p,
        })
        # device row for padded edge pos p: fwd = 1024*(p//512) + p%512
        pidx = np.arange(e_pad, dtype=np.int64)
        fwd_rows = 1024 * (pidx // 512) + pidx % 512
        og = np.full(m_pad, -1, dtype=np.int64)
        m = oge >= 0
        og[fwd_rows[m]] = oge[m]            # row ids 0..E-1
        og[fwd_rows[m] + 512] = e + oge[m]  # row ids E..2E-1
        orig_rows.append(og)

    key = (v, e_pad, tuple(plan))
    if key not in _neff_cache:
        _neff_cache[key] = _build_kernel(v, BANK, plan, e_pad)
    nc = _neff_cache[key]

    res = run_bass_kernel_spmd(nc, in_maps, list(range(N_CORES)), trace=_TRACE)
    last_result = res

    result = np.zeros(r, dtype=np.float32)
    for o, og in zip(res.results, orig_rows):
        m = og >= 0
        result[og[m]] = np.asarray(o["out"], dtype=np.float32).ravel()[m]
    return result
